# revision 7
# baseline (speedup 1.0000x reference)
"""CoarseMatcher (dual-softmax + mutual-NN matching) on 8 trn2 NeuronCores.

Sharding: core i handles batch b = i//2 and row-half h = i%2 of the
[L, S] confidence matrix (rows h*2400 .. h*2400+2400).

Math rewrite (verified vs reference to ~2.5e-6 rel):
    sim  = (f0 @ f1.T) / (sqrt(C)^2 * TEMP) = raw_dot / 25.6
    conf = softmax_L(sim) * softmax_S(sim)
         = exp(2*sim - ln(rowsum_exp) - ln(colsum_exp))
(no max-subtraction needed: |sim| <= ~4 for any sane input scale; exp is
safe in fp32 far beyond that).

Launch 1 (K1): per core, compute e = exp(sim) tile-by-tile; row sums via
ScalarE activation accum_out; column sums via a ones-vector matmul
(partition reduction on TensorE into PSUM).
Host: combine the two row-halves' column sums, take logs.
Launch 2 (K2): recompute sim, subtract ln(colsum) (VectorE, broadcast
tile), exp with scale=2/25.6 and bias=-ln(rowsum) (ScalarE), stream the
conf matrix to HBM.  Row-sums of conf (free via accum_out) serve as a
conservative detector for conf > THR hits (sum >= max): rows that fire
are resolved exactly on the host from the conf matrix it already holds.
"""

import numpy as np

B, L, S, C = 4, 4800, 4800, 256
LC = L // 2          # 2400 rows per core
P = 128
NB = (LC + P - 1) // P   # 19 row blocks (last is 96 rows)
CW = 480                 # column chunk (10 per 4800, fits one PSUM bank)
NC_CHUNKS = S // CW      # 10
KSCALE = 1.0 / 25.6      # sim = raw_dot * KSCALE
THR = 0.2
W0C = 80
H0C = 60
BORDER = 2
SCALE = 8.0

_cache = {}
_last_perf = []   # [K1 BassKernelResults, K2 BassKernelResults] of the last run


def _build_k1():
    import concourse.bacc as bacc
    import concourse.tile as tile
    import concourse.mybir as mybir

    nc = bacc.Bacc("TRN2", target_bir_lowering=False, debug=False,
                   enable_asserts=False, num_devices=8)
    dt = mybir.dt
    f0T = nc.dram_tensor("f0t", [C, LC], dt.float32r, kind="ExternalInput").ap()
    f1T = nc.dram_tensor("f1t", [C, S], dt.float32r, kind="ExternalInput").ap()
    rs_out = nc.dram_tensor("rowsum", [P, NB], dt.float32, kind="ExternalOutput").ap()
    cs_out = nc.dram_tensor("colsum", [S], dt.float32, kind="ExternalOutput").ap()

    with tile.TileContext(nc) as tc:
        with (
            tc.tile_pool(name="persist", bufs=1) as pp,
            tc.tile_pool(name="ework", bufs=4) as ep,
            tc.tile_pool(name="zp", bufs=4, space="PSUM") as zp,
            tc.tile_pool(name="csp", bufs=2, space="PSUM") as csp,
        ):
            # stationary / streaming operands, resident all kernel
            f0t_sb = [pp.tile([P, LC], dt.float32r, tag=f"f0t{k}", name=f"f0t{k}") for k in range(2)]
            f1t_sb = [pp.tile([P, S], dt.float32r, tag=f"f1t{k}", name=f"f1t{k}") for k in range(2)]
            for k in range(2):
                nc.sync.dma_start(out=f0t_sb[k][:, :], in_=f0T[k * P:(k + 1) * P, :])
                nc.sync.dma_start(out=f1t_sb[k][:, :], in_=f1T[k * P:(k + 1) * P, :])
            ones_f = pp.tile([P, 1], dt.float32, tag="ones_f", name="ones_f")
            nc.vector.memset(ones_f[:, :], 1.0)
            ones = pp.tile([P, 1], dt.float32r, tag="ones", name="ones")
            nc.vector.tensor_copy(ones[:, :], ones_f[:, :])
            # per row-block partial row sums (one col per column-chunk)
            rsparts = [pp.tile([P, NC_CHUNKS], dt.float32, tag=f"rsp{rb}", name=f"rsp{rb}")
                       for rb in range(NB)]
            rs_sb = pp.tile([P, NB], dt.float32, tag="rs_sb", name="rs_sb")
            cs_sb = pp.tile([1, S], dt.float32, tag="cs_sb", name="cs_sb")

            for cb in range(NC_CHUNKS):
                c0 = cb * CW
                cs_ps = csp.tile([1, CW], dt.float32, tag="cs_ps", name="cs_ps")
                for rb in range(NB):
                    m = min(P, LC - rb * P)
                    r0 = rb * P
                    z = zp.tile([P, CW], dt.float32, tag="z", name="z")
                    for k in range(2):
                        nc.tensor.matmul(
                            z[0:m, :],
                            f0t_sb[k][:, r0:r0 + m],
                            f1t_sb[k][:, c0:c0 + CW],
                            start=(k == 0), stop=(k == 1),
                        )
                    e = ep.tile([P, CW], dt.float32r, tag="e", name="e")
                    nc.scalar.activation(
                        e[0:m, :], z[0:m, :],
                        mybir.ActivationFunctionType.Exp,
                        scale=KSCALE,
                        accum_out=rsparts[rb][0:m, cb:cb + 1],
                    )
                    nc.tensor.matmul(
                        cs_ps[0:1, :],
                        ones[0:m, 0:1],
                        e[0:m, :],
                        start=(rb == 0), stop=(rb == NB - 1),
                        skip_group_check=True,
                    )
                nc.vector.tensor_copy(cs_sb[0:1, c0:c0 + CW], cs_ps[0:1, :])
            # reduce row-sum parts; ship small outputs
            for rb in range(NB):
                m = min(P, LC - rb * P)
                nc.vector.reduce_sum(
                    rs_sb[0:m, rb:rb + 1], rsparts[rb][0:m, :],
                    axis=mybir.AxisListType.X,
                )
            nc.sync.dma_start(out=rs_out[:, :], in_=rs_sb[:, :])
            nc.sync.dma_start(out=cs_out[None, :], in_=cs_sb[0:1, :])
    nc.compile()
    return nc


def _build_k2():
    import concourse.bacc as bacc
    import concourse.tile as tile
    import concourse.mybir as mybir

    nc = bacc.Bacc("TRN2", target_bir_lowering=False, debug=False,
                   enable_asserts=False, num_devices=8)
    dt = mybir.dt
    f0T = nc.dram_tensor("f0t", [C, LC], dt.float32r, kind="ExternalInput").ap()
    f1T = nc.dram_tensor("f1t", [C, S], dt.float32r, kind="ExternalInput").ap()
    # -ln(rowsum) laid out [128, NB] like K1's rowsum output
    nrc = nc.dram_tensor("negrcomb", [P, NB], dt.float32, kind="ExternalInput").ap()
    # ln(colsum) * 25.6/2  (pre-divided by the activation scale)
    cck = nc.dram_tensor("ccombk", [S], dt.float32, kind="ExternalInput").ap()
    conf_out = nc.dram_tensor("conf", [LC, S], dt.float32, kind="ExternalOutput").ap()
    rc_out = nc.dram_tensor("rowsumconf", [P, NB], dt.float32, kind="ExternalOutput").ap()

    K2SCALE = 2.0 * KSCALE

    with tile.TileContext(nc) as tc:
        with (
            tc.tile_pool(name="persist", bufs=1) as pp,
            tc.tile_pool(name="rowbuf", bufs=3) as rp,
            tc.tile_pool(name="zs", bufs=4) as zsp,
            tc.tile_pool(name="zp", bufs=6, space="PSUM") as zp,
        ):
            f0t_sb = [pp.tile([P, LC], dt.float32r, tag=f"f0t{k}", name=f"f0t{k}") for k in range(2)]
            f1t_sb = [pp.tile([P, S], dt.float32r, tag=f"f1t{k}", name=f"f1t{k}") for k in range(2)]
            for k in range(2):
                nc.sync.dma_start(out=f0t_sb[k][:, :], in_=f0T[k * P:(k + 1) * P, :])
                nc.sync.dma_start(out=f1t_sb[k][:, :], in_=f1T[k * P:(k + 1) * P, :])
            nrc_sb = pp.tile([P, NB], dt.float32, tag="nrc", name="nrc")
            nc.sync.dma_start(out=nrc_sb[:, :], in_=nrc[:, :])
            cck_row = pp.tile([1, S], dt.float32, tag="cck_row", name="cck_row")
            nc.sync.dma_start(out=cck_row[0:1, :], in_=cck[None, :])
            cck_bc = pp.tile([P, S], dt.float32, tag="cck_bc", name="cck_bc")
            nc.gpsimd.partition_broadcast(cck_bc[:, :], cck_row[0:1, :])

            rcparts = [pp.tile([P, NC_CHUNKS], dt.float32, tag=f"rcp{rb}", name=f"rcp{rb}")
                       for rb in range(NB)]
            rc_sb = pp.tile([P, NB], dt.float32, tag="rc_sb", name="rc_sb")

            for rb in range(NB):
                m = min(P, LC - rb * P)
                r0 = rb * P
                conf_row = rp.tile([P, S], dt.float32, tag="conf_row", name="conf_row")
                for cb in range(NC_CHUNKS):
                    c0 = cb * CW
                    z = zp.tile([P, CW], dt.float32, tag="z", name="z")
                    for k in range(2):
                        nc.tensor.matmul(
                            z[0:m, :],
                            f0t_sb[k][:, r0:r0 + m],
                            f1t_sb[k][:, c0:c0 + CW],
                            start=(k == 0), stop=(k == 1),
                        )
                    zs = zsp.tile([P, CW], dt.float32, tag="zs", name="zs")
                    # zs = raw_dot - ccomb/K2SCALE   (exact fp32 subtract)
                    nc.vector.tensor_sub(zs[0:m, :], z[0:m, :], cck_bc[0:m, c0:c0 + CW])
                    # conf = exp(K2SCALE*zs - rcomb)
                    nc.scalar.activation(
                        conf_row[0:m, c0:c0 + CW], zs[0:m, :],
                        mybir.ActivationFunctionType.Exp,
                        scale=K2SCALE,
                        bias=nrc_sb[0:m, rb:rb + 1],
                        accum_out=rcparts[rb][0:m, cb:cb + 1],
                    )
                nc.sync.dma_start(out=conf_out[r0:r0 + m, :], in_=conf_row[0:m, :])
            for rb in range(NB):
                m = min(P, LC - rb * P)
                nc.vector.reduce_sum(
                    rc_sb[0:m, rb:rb + 1], rcparts[rb][0:m, :],
                    axis=mybir.AxisListType.X,
                )
            nc.sync.dma_start(out=rc_out[:, :], in_=rc_sb[:, :])
    nc.compile()
    return nc


def _get_kernels():
    if "k1" not in _cache:
        _cache["k1"] = _build_k1()
        _cache["k2"] = _build_k2()
    return _cache["k1"], _cache["k2"]


def _unravel_rows(arr):
    """[128, NB] device layout -> [LC] flat (row rb*128+p at arr[p, rb])."""
    cols = [arr[0:min(P, LC - rb * P), rb] for rb in range(NB)]
    return np.concatenate(cols)


def _ravel_rows(vec):
    """[LC] flat -> [128, NB] device layout."""
    out = np.zeros((P, NB), dtype=vec.dtype)
    for rb in range(NB):
        m = min(P, LC - rb * P)
        out[0:m, rb] = vec[rb * P: rb * P + m]
    return out


def kernel(feat0, feat1):
    from concourse.bass_utils import run_bass_kernel_spmd

    feat0 = np.ascontiguousarray(feat0, dtype=np.float32)
    feat1 = np.ascontiguousarray(feat1, dtype=np.float32)
    k1, k2 = _get_kernels()
    core_ids = list(range(8))

    f1T = [np.ascontiguousarray(feat1[b].T) for b in range(B)]
    f0T = [np.ascontiguousarray(feat0[i // 2, (i % 2) * LC:(i % 2) * LC + LC].T)
           for i in range(8)]

    in1 = [{"f0t": f0T[i], "f1t": f1T[i // 2]} for i in range(8)]
    res1 = run_bass_kernel_spmd(k1, in1, core_ids=core_ids)
    r1 = res1.results

    # host combine: global per-batch column sums, logs
    in2 = []
    for i in range(8):
        b = i // 2
        rsum = _unravel_rows(r1[i]["rowsum"])            # [2400]
        csum = r1[2 * b]["colsum"] + r1[2 * b + 1]["colsum"]   # [4800]
        negrcomb = _ravel_rows(-np.log(rsum).astype(np.float32))
        ccombk = (np.log(csum) / (2.0 * KSCALE)).astype(np.float32)
        in2.append({"f0t": f0T[i], "f1t": f1T[b],
                    "negrcomb": negrcomb, "ccombk": ccombk})
    res2 = run_bass_kernel_spmd(k2, in2, core_ids=core_ids)
    r2 = res2.results
    _last_perf[:] = [res1, res2]

    conf = np.empty((B, L, S), dtype=np.float32)
    for i in range(8):
        conf[i // 2, (i % 2) * LC:(i % 2) * LC + LC] = r2[i]["conf"]

    # ---- matching outputs (host assembly; device supplied the detector) ----
    match = np.zeros((B, L), dtype=bool)
    s_ids = np.zeros((B, L), dtype=np.int32)
    mconf = np.zeros((B, L), dtype=np.float32)

    # conservative hit detector: rowsum(conf) >= rowmax(conf); no row can
    # have conf > THR unless its sum exceeds THR.
    i_h = np.arange(H0C)
    valid_h = (i_h >= BORDER) & (i_h < H0C - BORDER)
    i_w = np.arange(W0C)
    valid_w = (i_w >= BORDER) & (i_w < W0C - BORDER)
    valid_axis = (valid_h[:, None] & valid_w[None, :]).reshape(-1)  # [4800]

    for i in range(8):
        b, h = i // 2, i % 2
        rsc = _unravel_rows(r2[i]["rowsumconf"])
        hit_rows = np.nonzero(rsc > THR)[0]
        if hit_rows.size == 0:
            continue
        # exact slow path, replicating the reference's fp32 op order
        cb_full = conf[b]
        colmax = cb_full.max(axis=0)                       # [S]
        for r_local in hit_rows:
            l = h * LC + r_local
            row = cb_full[l]
            rowmax = row.max()
            mask_r = (row > THR) & valid_axis[l] & valid_axis
            score = (mask_r.astype(np.float32) * rowmax) * colmax
            sid = int(np.argmax(score))
            mm = score[sid]
            if mm > 0:
                match[b, l] = True
                s_ids[b, l] = sid
                mconf[b, l] = row[sid]

    l_idx = np.arange(L)
    mkpts0 = np.stack([l_idx % W0C, l_idx // W0C], axis=1).astype(np.float32) * SCALE
    mkpts1 = np.stack([s_ids % W0C, s_ids // W0C], axis=-1).astype(np.float32) * SCALE

    return conf, match, s_ids, mconf, mkpts0, mkpts1
